# revision 11
# baseline (speedup 1.0000x reference)
"""Trainium2 Bass kernel for nn_MetricBiasUpdater.

Computes, for H [4,2048,1024], B_prev [4,2048,2048], W [32,1024]:
    G    = H @ W.T                                   [4,2048,32]
    dist = |G_i|^2 + |G_j|^2 - 2 G_i.G_j             [4,2048,2048]
    out  = clip(alpha*B_prev - beta*max(dist,0), -10, 10)

Sharding: 8 cores = (batch b, row-half h).  Core (b,h) computes output rows
[h*1024,(h+1)*1024) of batch b for all 2048 columns.  To keep the SPMD
program identical on every core, the host hands each core H[b]^T with the
columns rotated so the core's own 1024 rows come first; B_prev columns are
rotated the same way and the result columns are rotated back on the host.

On-core algorithm: one augmented matmul produces -beta*dist directly:
    lhsT = -beta * [G_i; |G_i|^2; 1]   (K padded 34 -> 128 with zeros)
    rhs  =         [-2*G_j; 1; |G_j|^2]
    psum[i,j] = sum_k lhsT[k,i]*rhs[k,j] = -beta*dist[i,j]
then on the vector engine:
    t = min(psum, 0) + alpha*B_prev      == alpha*B_prev - beta*max(dist,0)
    o = max(min(t, 10), -10)

SBUF partition-offset rule: sub-128-partition accesses must start at a
multiple of 32, so the two augmentation rows live at partitions 32 and 64
(rows 33..63 and 65..127 stay zero and contribute nothing to the matmul).
"""

import os
import sys

# The bass runtime drives the NeuronCores through the jax "axon" PJRT
# platform.  If a caller pinned JAX_PLATFORMS to cpu (common for running
# the pure-jax reference), undo that before jax is first imported.
if "jax" not in sys.modules:
    _jp = os.environ.get("JAX_PLATFORMS")
    if _jp is not None and "axon" not in _jp and "neuron" not in _jp:
        del os.environ["JAX_PLATFORMS"]

sys.path.insert(0, "/opt/trn_rl_repo")

import numpy as np

import concourse.bass as bass
import concourse.bacc as bacc
import concourse.mybir as mybir
from concourse.tile import TileContext
from concourse.bass_utils import run_bass_kernel_spmd

F32 = mybir.dt.float32
BF16 = mybir.dt.bfloat16
AF = mybir.ActivationFunctionType
ALU = mybir.AluOpType

B, N, D, K = 4, 2048, 1024, 32
HALF = N // 2            # rows per core
CLAMP = 10.0
N_CORES = 8
P = 128                  # partitions
JT = 512                 # moving free dim per matmul
NJ = N // JT             # 4 column chunks
KC = D // P              # 8 contraction chunks for G

_nc_cache: dict = {}


def _build_nc(alpha: float, beta: float) -> "bass.Bass":
    # Bacc (not raw Bass): its finalize() runs the legalization passes that
    # split multi-sem waits (PE instructions have a single wait slot).
    nc = bacc.Bacc(None)
    ht = nc.dram_tensor("ht", [D, N], F32, kind="ExternalInput")
    wt = nc.dram_tensor("wt", [D, K], F32, kind="ExternalInput")
    bp_in = nc.dram_tensor("bprev", [HALF, N], F32, kind="ExternalInput")
    out = nc.dram_tensor("out", [HALF, N], F32, kind="ExternalOutput")
    nb = -float(beta)

    with TileContext(nc) as tc:
        with tc.tile_pool(name="persist", bufs=1) as persist:
            # All matmul operands are bf16 (PE runs fp32 at 1/4 rate); PSUM
            # accumulation stays fp32, and B_prev/output stay fp32, so the
            # only precision loss is on the tiny -beta*dist term.
            # W^T in [128, KC, K] layout: wt_sb[p, c, k] = W[k, c*128+p]
            wt_sb = persist.tile([P, KC, K], BF16)
            nc.gpsimd.dma_start(
                out=wt_sb[:], in_=wt.rearrange("(c p) k -> p c k", p=P)
            )
            ones_sb = persist.tile([K, 1], BF16)
            nc.vector.memset(ones_sb[:], 1.0)

            # Augmented operands for the dist matmul (K padded to 128).
            # Contraction pairing: rows 0..31 G-dot term, row 32 gsq_i term,
            # row 64 gsq_j term (offsets must be multiples of 32).
            R1, R2 = 32, 64
            rhs_aug = persist.tile([P, N], BF16)     # rows: -2G | 1 | gsq
            lhs_aug = persist.tile([P, HALF], BF16)  # rows: -b*G | -b*gsq | -b
            gsq_in = persist.tile([K, N], BF16)      # G^2 (for the gsq matmul)
            nc.vector.memset(rhs_aug[:], 0.0)
            nc.vector.memset(lhs_aug[:], 0.0)
            nc.vector.memset(rhs_aug[R1 : R1 + 1, :], 1.0)
            nc.vector.memset(lhs_aug[R2 : R2 + 1, :], nb)

            # ---------------- G phase ----------------
            with (
                tc.tile_pool(name="hpool", bufs=KC) as hp,
                tc.tile_pool(name="gpsum", bufs=2, space="PSUM") as gp,
                tc.tile_pool(name="qpsum", bufs=2, space="PSUM") as qp,
            ):
                htr = ht.rearrange("(c p) j -> c p j", p=P)
                hts = []
                for kc in range(KC):
                    t = hp.tile([P, N], BF16, tag="ht")
                    # gpsimd (SWDGE) casts f32 -> bf16 in the DMA datapath.
                    nc.gpsimd.dma_start(out=t[:], in_=htr[kc])
                    hts.append(t)
                for jc in range(NJ):
                    js = slice(jc * JT, (jc + 1) * JT)
                    pg = gp.tile([K, JT], F32, tag="pg")
                    for kc in range(KC):
                        nc.tensor.matmul(
                            pg[:],
                            wt_sb[:, kc, :],
                            hts[kc][:, js],
                            start=(kc == 0),
                            stop=(kc == KC - 1),
                        )
                    # G^T chunk is in pg.  Write the scaled copies.
                    nc.scalar.activation(rhs_aug[0:K, js], pg[:], AF.Copy, scale=-2.0)
                    if jc * JT < HALF:
                        nc.scalar.activation(lhs_aug[0:K, js], pg[:], AF.Copy, scale=nb)
                    nc.scalar.activation(gsq_in[:, js], pg[:], AF.Square)
                for jc in range(NJ):
                    js = slice(jc * JT, (jc + 1) * JT)
                    pq = qp.tile([1, JT], F32, tag="pq")
                    nc.tensor.matmul(
                        pq[:], ones_sb[:], gsq_in[:, js], start=True, stop=True
                    )
                    nc.scalar.activation(rhs_aug[R2 : R2 + 1, js], pq[:], AF.Copy)
                    if jc * JT < HALF:
                        nc.scalar.activation(
                            lhs_aug[R1 : R1 + 1, js], pq[:], AF.Copy, scale=nb
                        )

            # ---------------- dist + EMA phase ----------------
            with (
                tc.tile_pool(name="dpsum", bufs=2, space="PSUM") as dp,
                tc.tile_pool(name="bpool", bufs=8) as bpool,
                tc.tile_pool(name="opool", bufs=3) as opool,
            ):
                for it in range(HALF // P):  # 8 i-tiles of 128 rows
                    isl = slice(it * P, (it + 1) * P)
                    pd = dp.tile([P, N], F32, tag="pd")
                    for jc in range(NJ):
                        js = slice(jc * JT, (jc + 1) * JT)
                        nc.tensor.matmul(
                            pd[:, js],
                            lhs_aug[:, isl],
                            rhs_aug[:, js],
                            start=True,
                            stop=True,
                        )
                    bt = bpool.tile([P, N], F32, tag="bt")
                    nc.sync.dma_start(out=bt[:], in_=bp_in[isl, :])
                    if alpha != 1.0:
                        nc.vector.tensor_scalar_mul(bt[:], bt[:], float(alpha))
                    tt = opool.tile([P, N], F32, tag="tt")
                    nc.vector.scalar_tensor_tensor(
                        tt[:], pd[:], 0.0, bt[:], ALU.min, ALU.add
                    )
                    ot = opool.tile([P, N], F32, tag="ot")
                    nc.vector.tensor_scalar(
                        ot[:], tt[:], CLAMP, -CLAMP, ALU.min, ALU.max
                    )
                    nc.sync.dma_start(out=out[isl, :], in_=ot[:])
    if not nc.is_finalized():
        nc.finalize()
    return nc


def _get_nc(alpha: float, beta: float) -> "bass.Bass":
    key = (alpha, beta)
    if key not in _nc_cache:
        _nc_cache[key] = _build_nc(alpha, beta)
    return _nc_cache[key]


def _make_in_maps(H, B_prev, W):
    wt_host = np.ascontiguousarray(W.T)  # [1024, 32]
    in_maps = []
    for c in range(N_CORES):
        bidx, h = divmod(c, 2)
        htb = H[bidx].T  # [1024, 2048]
        bp = B_prev[bidx, h * HALF : (h + 1) * HALF, :]
        if h == 1:
            htb = np.concatenate([htb[:, HALF:], htb[:, :HALF]], axis=1)
            bp = np.concatenate([bp[:, HALF:], bp[:, :HALF]], axis=1)
        in_maps.append(
            {
                "ht": np.ascontiguousarray(htb),
                "wt": wt_host,
                "bprev": np.ascontiguousarray(bp),
            }
        )
    return in_maps


def _assemble(results) -> np.ndarray:
    out = np.empty((B, N, N), np.float32)
    for c in range(N_CORES):
        bidx, h = divmod(c, 2)
        r = results[c]["out"]
        if h == 1:
            r = np.concatenate([r[:, HALF:], r[:, :HALF]], axis=1)
        out[bidx, h * HALF : (h + 1) * HALF, :] = r
    return out


def _run(H, B_prev, W, alpha, beta, **rbk_kwargs):
    H = np.ascontiguousarray(np.asarray(H, dtype=np.float32))
    B_prev = np.ascontiguousarray(np.asarray(B_prev, dtype=np.float32))
    W = np.ascontiguousarray(np.asarray(W, dtype=np.float32))
    nc = _get_nc(float(alpha), float(beta))
    in_maps = _make_in_maps(H, B_prev, W)
    res = run_bass_kernel_spmd(nc, in_maps, list(range(N_CORES)), **rbk_kwargs)
    return _assemble(res.results), res


def kernel(H, B_prev, W, alpha, beta) -> np.ndarray:
    out, _ = _run(H, B_prev, W, alpha, beta)
    return out


# revision 22
# speedup vs baseline: 1425.6444x; 1425.6444x over previous
"""Trainium2 Bass kernel for nn_MetricBiasUpdater.

Computes, for H [4,2048,1024], B_prev [4,2048,2048], W [32,1024]:
    G    = H @ W.T                                   [4,2048,32]
    dist = |G_i|^2 + |G_j|^2 - 2 G_i.G_j             [4,2048,2048]
    out  = clip(alpha*B_prev - beta*max(dist,0), -10, 10)

Sharding: 8 cores = (batch b, row-half h).  Core (b,h) computes output rows
[h*1024,(h+1)*1024) of batch b for all 2048 columns.  To keep the SPMD
program identical on every core, the host hands each core H[b]^T with the
columns rotated so the core's own 1024 rows come first; B_prev columns are
rotated the same way and the result columns are rotated back on the host.

On-core algorithm: one augmented matmul produces -beta*dist directly:
    lhsT = -beta * [G_i; |G_i|^2; 1]   (K padded 34 -> 128 with zeros)
    rhs  =         [-2*G_j; 1; |G_j|^2]
    psum[i,j] = sum_k lhsT[k,i]*rhs[k,j] = -beta*dist[i,j]
then on the vector engine:
    t = min(psum, 0) + alpha*B_prev      == alpha*B_prev - beta*max(dist,0)
    o = max(min(t, 10), -10)

All matmul operands are bf16 (PE runs fp32 at 1/4 rate); PSUM accumulation
stays fp32, and B_prev / the output stay fp32, so the only precision loss is
on the tiny -beta*dist term (abs err ~3e-5 on this data).

SBUF partition-offset rule: sub-128-partition accesses must start at a
multiple of 32, so the two augmentation rows live at partitions 32 and 64
(rows 33..63 and 65..127 stay zero and contribute nothing to the matmul).
"""

import os
import sys

# The bass runtime drives the NeuronCores through the jax "axon" PJRT
# platform.  If a caller pinned JAX_PLATFORMS to cpu (common for running
# the pure-jax reference), undo that before jax is first imported.
if "jax" not in sys.modules:
    _jp = os.environ.get("JAX_PLATFORMS")
    if _jp is not None and "axon" not in _jp and "neuron" not in _jp:
        del os.environ["JAX_PLATFORMS"]

sys.path.insert(0, "/opt/trn_rl_repo")

import numpy as np

import concourse.bass as bass
import concourse.bacc as bacc
import concourse.mybir as mybir
from concourse.tile import TileContext
from concourse.bass_utils import run_bass_kernel_spmd

F32 = mybir.dt.float32
BF16 = mybir.dt.bfloat16
AF = mybir.ActivationFunctionType
ALU = mybir.AluOpType

B, N, D, K = 4, 2048, 1024, 32
HALF = N // 2            # rows per core
CLAMP = 10.0
N_CORES = 8
P = 128                  # partitions
JT = 512                 # moving free dim per matmul
NJ = N // JT             # 4 column chunks
KC = D // P              # 8 contraction chunks for G
R1, R2 = 32, 64          # augmentation rows (must be multiples of 32)

# D-split mode: each core of a (b,0)/(b,1) pair reads only half of H[b]^T
# (split along the d contraction axis), computes a partial G, and the pair
# AllReduces the small [32, 2048] G before the dist phase.  Halves the H
# traffic (8 -> 4 MiB per core).  The core's own row-half of G is then
# selected with a partition-id-driven dynamic slice (no host-side column
# rotation in this mode).
DSPLIT = os.environ.get("KERNEL_DSPLIT", "1") != "0"
D2 = D // 2

_nc_cache: dict = {}


def _build_nc(alpha: float, beta: float, loop_reps: int | None = None) -> "bass.Bass":
    # Bacc (not raw Bass): its finalize() runs the legalization passes that
    # split multi-sem waits (PE instructions have a single wait slot).
    nc = bacc.Bacc(None, num_devices=N_CORES)
    d_in = D2 if DSPLIT else D
    ht = nc.dram_tensor("ht", [d_in, N], F32, kind="ExternalInput")
    wt = nc.dram_tensor("wt", [d_in, K], F32, kind="ExternalInput")
    bp_in = nc.dram_tensor("bprev", [HALF, N], F32, kind="ExternalInput")
    out = nc.dram_tensor("out", [HALF, N], F32, kind="ExternalOutput")

    with TileContext(nc) as tc:
        # Pools are shared across benchmark reps so PSUM/SBUF slot reuse
        # carries proper cross-rep dependencies (separate pools would alias
        # the same PSUM banks with no ordering).
        # PSUM budget: gp 2 + qp 2 + dp 2*2 = 8 banks.
        with (
            tc.tile_pool(name="persist", bufs=1) as persist,
            tc.tile_pool(name="hpool", bufs=d_in // P) as hp,
            tc.tile_pool(name="gpsum", bufs=2, space="PSUM") as gp,
            tc.tile_pool(name="qpsum", bufs=2, space="PSUM") as qp,
            tc.tile_pool(name="dpsum", bufs=2, space="PSUM") as dp,
            tc.tile_pool(name="bpool", bufs=8) as bpool,
            tc.tile_pool(name="opool", bufs=3) as opool,
            tc.tile_pool(name="drampool", bufs=1, space="DRAM") as drampool,
        ):
            pools = dict(
                persist=persist, hp=hp, gp=gp, qp=qp, dp=dp, bpool=bpool,
                opool=opool, drampool=drampool,
            )
            for _ in range(loop_reps or 1):
                _emit_body(nc, tc, pools, ht, wt, bp_in, out, alpha, beta)
    if not nc.is_finalized():
        nc.finalize()
    return nc


def _emit_body(nc, tc, pools, ht, wt, bp_in, out, alpha: float, beta: float):
    nb = -float(beta)
    persist, hp, gp, qp, dp = (
        pools["persist"],
        pools["hp"],
        pools["gp"],
        pools["qp"],
        pools["dp"],
    )
    bpool, opool = pools["bpool"], pools["opool"]

    # W^T in [128, n_chunks, K] layout: wt_sb[p, c, k] = W^T[c*128+p, k]
    wt_sb = persist.tile([P, (D2 if DSPLIT else D) // P, K], BF16, tag="wt_sb")
    nc.gpsimd.dma_start(out=wt_sb[:], in_=wt.rearrange("(c p) k -> p c k", p=P))
    ones_sb = persist.tile([K, 1], BF16, tag="ones_sb")
    nc.vector.memset(ones_sb[:], 1.0)

    # Augmented operands for the dist matmul (K padded to 128).
    # Contraction pairing: rows 0..31 G-dot term, row R1 gsq_i term,
    # row R2 gsq_j term.
    rhs_aug = persist.tile([P, N], BF16, tag="rhs_aug")   # rows: -2G | 1 | gsq
    lhs_aug = persist.tile([P, HALF], BF16, tag="lhs_aug")  # -b*G | -b*gsq | -b
    gsq_in = persist.tile([K, N], BF16, tag="gsq_in")     # G^2
    nc.vector.memset(rhs_aug[:], 0.0)
    nc.vector.memset(lhs_aug[:], 0.0)
    nc.vector.memset(rhs_aug[R1 : R1 + 1, :], 1.0)
    nc.vector.memset(lhs_aug[R2 : R2 + 1, :], nb)

    # ---------------- G phase ----------------
    kc_n = (D2 if DSPLIT else D) // P
    htr = ht.rearrange("(c p) j -> c p j", p=P)
    hts = []
    for kc in range(kc_n):
        t = hp.tile([P, N], BF16, tag="ht")
        # gpsimd (SWDGE) casts f32 -> bf16 in the DMA datapath.
        nc.gpsimd.dma_start(out=t[:], in_=htr[kc])
        hts.append(t)

    if not DSPLIT:
        for jc in range(NJ):
            js = slice(jc * JT, (jc + 1) * JT)
            pg = gp.tile([K, JT], F32, tag="pg")
            for kc in range(kc_n):
                nc.tensor.matmul(
                    pg[:],
                    wt_sb[:, kc, :],
                    hts[kc][:, js],
                    start=(kc == 0),
                    stop=(kc == kc_n - 1),
                )
            # G^T chunk is in pg.  Write the scaled copies.
            nc.scalar.activation(rhs_aug[0:K, js], pg[:], AF.Copy, scale=-2.0)
            if jc * JT < HALF:
                nc.scalar.activation(lhs_aug[0:K, js], pg[:], AF.Copy, scale=nb)
            nc.scalar.activation(gsq_in[:, js], pg[:], AF.Square)
        for jc in range(NJ):
            js = slice(jc * JT, (jc + 1) * JT)
            pq = qp.tile([1, JT], F32, tag="pq")
            nc.tensor.matmul(pq[:], ones_sb[:], gsq_in[:, js], start=True, stop=True)
            nc.scalar.activation(rhs_aug[R2 : R2 + 1, js], pq[:], AF.Copy)
            if jc * JT < HALF:
                nc.scalar.activation(
                    lhs_aug[R1 : R1 + 1, js], pq[:], AF.Copy, scale=nb
                )
    else:
        # Partial G from this core's d-half, staged to DRAM, pair-AllReduced,
        # then reloaded.  gpart_sb holds the f32 partial G^T.
        gpart_sb = persist.tile([K, N], F32, tag="gpart_sb")
        gfull_sb = persist.tile([K, N], F32, tag="gfull_sb")
        gsqf_sb = persist.tile([1, N], F32, tag="gsqf_sb")
        drampool = pools["drampool"]
        gpart_d = drampool.tile([K, N], F32, tag="gpart_d")
        gfull_d = drampool.tile([K, N], F32, tag="gfull_d")
        for jc in range(NJ):
            js = slice(jc * JT, (jc + 1) * JT)
            pg = gp.tile([K, JT], F32, tag="pg")
            for kc in range(kc_n):
                nc.tensor.matmul(
                    pg[:],
                    wt_sb[:, kc, :],
                    hts[kc][:, js],
                    start=(kc == 0),
                    stop=(kc == kc_n - 1),
                )
            nc.scalar.activation(gpart_sb[:, js], pg[:], AF.Copy)
        nc.sync.dma_start(out=gpart_d[:], in_=gpart_sb[:])
        nc.gpsimd.collective_compute(
            "AllReduce",
            ALU.add,
            replica_groups=[[2 * i, 2 * i + 1] for i in range(N_CORES // 2)],
            ins=[gpart_d[:]],
            outs=[gfull_d[:]],
        )
        nc.sync.dma_start(out=gfull_sb[:], in_=gfull_d[:])
        # Build the augmented operands from the reduced G.
        nc.scalar.activation(rhs_aug[0:K, :], gfull_sb[:], AF.Copy, scale=-2.0)
        nc.scalar.activation(gsq_in[:], gfull_sb[:], AF.Square)
        roff = (nc.scalar.partition_id() & 1) * HALF
        nc.scalar.activation(
            lhs_aug[0:K, 0:HALF],
            gfull_sb[:, bass.ds(roff, HALF)],
            AF.Copy,
            scale=nb,
        )
        for jc in range(NJ):
            js = slice(jc * JT, (jc + 1) * JT)
            pq = qp.tile([1, JT], F32, tag="pq")
            nc.tensor.matmul(pq[:], ones_sb[:], gsq_in[:, js], start=True, stop=True)
            nc.scalar.activation(rhs_aug[R2 : R2 + 1, js], pq[:], AF.Copy)
            nc.scalar.activation(gsqf_sb[:, js], pq[:], AF.Copy)
        nc.scalar.activation(
            lhs_aug[R1 : R1 + 1, 0:HALF],
            gsqf_sb[:, bass.ds(roff, HALF)],
            AF.Copy,
            scale=nb,
        )

    # ---------------- dist + EMA phase ----------------
    for it in range(HALF // P):  # 8 i-tiles of 128 rows
        isl = slice(it * P, (it + 1) * P)
        bt = bpool.tile([P, N], F32, tag="bt")
        nc.sync.dma_start(out=bt[:], in_=bp_in[isl, :])
        if alpha != 1.0:
            nc.vector.tensor_scalar_mul(bt[:], bt[:], float(alpha))
        tt = opool.tile([P, N], F32, tag="tt")
        for hh in range(2):  # dist psum in two [128, 1024] pieces (2 banks each)
            hs = slice(hh * (N // 2), (hh + 1) * (N // 2))
            pd = dp.tile([P, N // 2], F32, tag="pd")
            for jc2 in range(2):
                jl = slice(jc2 * JT, (jc2 + 1) * JT)
                jg = slice(hh * (N // 2) + jc2 * JT, hh * (N // 2) + (jc2 + 1) * JT)
                nc.tensor.matmul(
                    pd[:, jl], lhs_aug[:, isl], rhs_aug[:, jg], start=True, stop=True
                )
            nc.vector.scalar_tensor_tensor(
                tt[:, hs], pd[:], 0.0, bt[:, hs], ALU.min, ALU.add
            )
        ot = opool.tile([P, N], F32, tag="ot")
        nc.vector.tensor_scalar(ot[:], tt[:], CLAMP, -CLAMP, ALU.min, ALU.max)
        nc.sync.dma_start(out=out[isl, :], in_=ot[:])


def _get_nc(alpha: float, beta: float) -> "bass.Bass":
    key = (alpha, beta)
    if key not in _nc_cache:
        _nc_cache[key] = _build_nc(alpha, beta)
    return _nc_cache[key]


def _make_in_maps(H, B_prev, W):
    wt_host = np.ascontiguousarray(W.T)  # [1024, 32]
    in_maps = []
    for c in range(N_CORES):
        bidx, h = divmod(c, 2)
        htb = H[bidx].T  # [1024, 2048]
        bp = B_prev[bidx, h * HALF : (h + 1) * HALF, :]
        if DSPLIT:
            # natural column order; this core reads only its d-half
            htb = htb[h * D2 : (h + 1) * D2]
            wt_c = wt_host[h * D2 : (h + 1) * D2]
        else:
            wt_c = wt_host
            if h == 1:
                htb = np.concatenate([htb[:, HALF:], htb[:, :HALF]], axis=1)
                bp = np.concatenate([bp[:, HALF:], bp[:, :HALF]], axis=1)
        in_maps.append(
            {
                "ht": np.ascontiguousarray(htb),
                "wt": np.ascontiguousarray(wt_c),
                "bprev": np.ascontiguousarray(bp),
            }
        )
    return in_maps


def _assemble(results) -> np.ndarray:
    out = np.empty((B, N, N), np.float32)
    for c in range(N_CORES):
        bidx, h = divmod(c, 2)
        r = results[c]["out"]
        if not DSPLIT and h == 1:
            r = np.concatenate([r[:, HALF:], r[:, :HALF]], axis=1)
        out[bidx, h * HALF : (h + 1) * HALF, :] = r
    return out


def _run(H, B_prev, W, alpha, beta, **rbk_kwargs):
    H = np.ascontiguousarray(np.asarray(H, dtype=np.float32))
    B_prev = np.ascontiguousarray(np.asarray(B_prev, dtype=np.float32))
    W = np.ascontiguousarray(np.asarray(W, dtype=np.float32))
    nc = _get_nc(float(alpha), float(beta))
    in_maps = _make_in_maps(H, B_prev, W)
    res = run_bass_kernel_spmd(nc, in_maps, list(range(N_CORES)), **rbk_kwargs)
    return _assemble(res.results), res


def kernel(H, B_prev, W, alpha, beta) -> np.ndarray:
    out, _ = _run(H, B_prev, W, alpha, beta)
    return out
